# revision 12
# baseline (speedup 1.0000x reference)
"""AdaAttention Trainium2 kernel — data-parallel over batch across 8 NeuronCores.

Full shapes: h [1024,512], sentinel [1024,512], att_feats [1024,96,2048] -> out [1024,512].
Per core: b=128 batch rows. Token axis x = s*128 + b (s-major), N_tok = 12288.

v2: fp8 DoubleRow matmuls + online exp-weighted cHat (no softmax barrier, no spill).
  att_feats --SWDGE cast f32->fp8--> nat[b,4,2048] --u16 xbar--> attf[q,fb,i,j] (fp8 pairs)
  MM1 (fp8 DR): attT[r,x] = relu(ps/128 + b_ae)  [w_ae prescaled x128]
  MM2 (fp8 DR): ps2 = 64*att_embd ; DVE += h_eT' (64*h_e) ; ACT tanh(x/64 + b_c) -> hat fp8
  logits (fp8 DR, w_al x16): ps_l row ; ACT exp(x/16) -> e_row bf16 (no max-sub, |logit|<=12)
  e_row --K=1 ones matmul--> e_rep psum ; DVE: cacc += reduce_i(attT * e_rep)   (cHatT unnorm)
  e_row --PE col transpose--> e_sb[b, 97] table
  tail: ssum=reduce(e_sb); rinv; transpose+bcast; attenT=(cacc*rinv_rep)+hT_bf;
        out = tanh(attenT @ w_o + b_o) via PE; transpose; store.
fp8 f-mapping through the u16 xbar: f = 256*fb + 2*p + parity, host-permutes weights to match.
"""
import sys

for p in ("/opt/trn_rl_repo", "/opt/pypackages"):
    if p not in sys.path:
        sys.path.insert(0, p)

import numpy as np
import ml_dtypes
from contextlib import ExitStack

import concourse.bass as bass
import concourse.bacc as bacc
import concourse.mybir as mybir
from concourse import tile

F32 = mybir.dt.float32
BF16 = mybir.dt.bfloat16
F8 = mybir.dt.float8e4
AF = mybir.ActivationFunctionType
ALU = mybir.AluOpType
DR = mybir.MatmulPerfMode.DoubleRow

NCORES = 8
B_LOC = 128          # batch rows per core
S = 96               # attention slots
F = 2048             # att feature size
R = 512              # rnn size
A = 512              # att hidden size
NTOK = B_LOC * S     # 12288
XCHUNK = 512         # tokens per pipeline chunk (4 s-tiles)
NCHUNKS = NTOK // XCHUNK       # 24
S_PER_CHUNK = XCHUNK // B_LOC  # 4
FB = F // 256        # 8 f-pair-blocks
RT = R // 128        # 4
AT = A // 128        # 4

W_AE_SCALE = 128.0
W_C_SCALE = 64.0
W_AL_SCALE = 16.0


def build_nc():
    nc = bacc.Bacc("TRN2", target_bir_lowering=False, debug=False)

    # ---- DRAM parameters (per-core shard shapes) ----
    att_feats = nc.declare_dram_parameter("att_feats", [B_LOC, S, F], F32, isOutput=False)
    h_in = nc.declare_dram_parameter("h", [B_LOC, R], F32, isOutput=False)
    sent_in = nc.declare_dram_parameter("sentinel", [B_LOC, R], F32, isOutput=False)
    # fp8 weights, host-permuted for DoubleRow (see prep_shared)
    w_ae_d = nc.declare_dram_parameter("w_ae", [128, FB, 2, R], F8, isOutput=False)
    w_c_d = nc.declare_dram_parameter("w_c", [128, 2, 2, A], F8, isOutput=False)
    w_s_d = nc.declare_dram_parameter("w_s", [128, 2, 2, A], F8, isOutput=False)
    w_h_d = nc.declare_dram_parameter("w_h", [128, 2, 2, A], F8, isOutput=False)
    w_al_d = nc.declare_dram_parameter("w_al", [128, 2, 2, 16], F8, isOutput=False)
    w_o_d = nc.declare_dram_parameter("w_o", [128, RT, R], BF16, isOutput=False)
    b_ae_d = nc.declare_dram_parameter("b_ae", [128, RT], F32, isOutput=False)
    b_c_d = nc.declare_dram_parameter("b_c", [128, AT], F32, isOutput=False)
    b_s_d = nc.declare_dram_parameter("b_s", [128, AT], F32, isOutput=False)
    b_h64_d = nc.declare_dram_parameter("b_h64", [128, AT], F32, isOutput=False)
    b_o_d = nc.declare_dram_parameter("b_o", [128, RT], F32, isOutput=False)
    ident_d = nc.declare_dram_parameter("ident", [128, 128], BF16, isOutput=False)
    ident_f32_d = nc.declare_dram_parameter("ident_f32", [128, 128], F32, isOutput=False)
    ones_d = nc.declare_dram_parameter("ones_row", [1, 128], BF16, isOutput=False)
    out_d = nc.declare_dram_parameter("out", [B_LOC, R], F32, isOutput=True)

    with tile.TileContext(nc) as tc, ExitStack() as ctx:
        # ---- pools ----
        cp = ctx.enter_context(tc.tile_pool(name="consts", bufs=1))
        nat_p = ctx.enter_context(tc.tile_pool(name="nat", bufs=4))
        attf_p = ctx.enter_context(tc.tile_pool(name="attf", bufs=3))
        attT_p = ctx.enter_context(tc.tile_pool(name="attT", bufs=6))
        hat_p = ctx.enter_context(tc.tile_pool(name="hat", bufs=3))
        stg_p = ctx.enter_context(tc.tile_pool(name="stg", bufs=2))
        prod_p = ctx.enter_context(tc.tile_pool(name="prod", bufs=2))
        erow_p = ctx.enter_context(tc.tile_pool(name="erow", bufs=3))
        small_p = ctx.enter_context(tc.tile_pool(name="small", bufs=2))
        soft_p = ctx.enter_context(tc.tile_pool(name="soft", bufs=3))
        ps_mm1 = ctx.enter_context(tc.tile_pool(name="ps_mm1", bufs=2, space="PSUM"))
        ps_mm2 = ctx.enter_context(tc.tile_pool(name="ps_mm2", bufs=2, space="PSUM"))
        ps_rep = ctx.enter_context(tc.tile_pool(name="ps_rep", bufs=2, space="PSUM"))
        ps_small = ctx.enter_context(tc.tile_pool(name="ps_small", bufs=2, space="PSUM"))

        nat_tiles = {}

        def stage_in(c):
            # plain f32 loads (SWDGE-cast DMAs only reach ~160GB/s; plain are
            # full rate). One big 3-slice SWDGE DMA (contiguous 24KB/partition)
            # + 1 slice on HWDGE; f32->fp8 casts spread over DVE/ACT/GpSimd.
            s0 = c * S_PER_CHUNK
            nat = nat_p.tile([B_LOC, S_PER_CHUNK, F], F8, tag="nat", name=f"nat_{c}")
            stg3 = stg_p.tile([B_LOC, 3, F], F32, tag="stg3", name=f"stg3_{c}")
            nc.gpsimd.dma_start(out=stg3[:], in_=att_feats[:, s0:s0 + 3, :])
            stg1 = stg_p.tile([B_LOC, F], F32, tag="stg1", name=f"stg1_{c}")
            nc.scalar.dma_start(out=stg1[:], in_=att_feats[:, s0 + 3, :])
            nc.vector.tensor_copy(nat[:, 0, :], stg3[:, 0, :])
            nc.vector.tensor_copy(nat[:, 1, :], stg3[:, 1, :])
            nc.scalar.activation(nat[:, 2, :], stg3[:, 2, :], AF.Copy)
            nc.gpsimd.tensor_copy(nat[:, 3, :], stg1[:])
            nat_tiles[c] = nat

        def const_tile(name, shape, dtype, src):
            t = cp.tile(shape, dtype, tag=name, name=name)
            nc.scalar.dma_start(out=t[:], in_=src[:])
            return t

        # w_ae + small consts first so chunk-0 matmuls can start ASAP
        w_ae = const_tile("w_ae", [128, FB, 2, R], F8, w_ae_d)
        b_ae = const_tile("b_ae", [128, RT], F32, b_ae_d)
        ident = const_tile("ident", [128, 128], BF16, ident_d)
        ident_f32 = const_tile("ident_f32", [128, 128], F32, ident_f32_d)
        ones_row = const_tile("ones_row", [1, 128], BF16, ones_d)

        stage_in(0)
        stage_in(1)

        # ---- remaining constants / weights ----
        w_c = const_tile("w_c", [128, 2, 2, A], F8, w_c_d)
        w_s = const_tile("w_s", [128, 2, 2, A], F8, w_s_d)
        w_h = const_tile("w_h", [128, 2, 2, A], F8, w_h_d)
        w_o = const_tile("w_o", [128, RT, R], BF16, w_o_d)
        wal = const_tile("wal", [128, 2, 2, 16], F8, w_al_d)
        b_c = const_tile("b_c", [128, AT], F32, b_c_d)
        b_s = const_tile("b_s", [128, AT], F32, b_s_d)
        b_h64 = const_tile("b_h64", [128, AT], F32, b_h64_d)
        b_o = const_tile("b_o", [128, RT], F32, b_o_d)

        # exp'd-logit table [b, 1+S] f32 and the cHat accumulator [r_p, rb, b] f32
        e_sb = cp.tile([B_LOC, 1 + S], F32, tag="e_sb", name="e_sb")
        cacc = cp.tile([128, RT, B_LOC], F32, tag="cacc", name="cacc")

        prep_out = {}

        def prep():
            # ---- h / sentinel prep ----
            h_bf = cp.tile([B_LOC, R], BF16, tag="h_bf", name="h_bf")
            nc.gpsimd.dma_start(out=h_bf[:], in_=h_in[:])  # cast f32->bf16 in DMA
            sent_bf = cp.tile([B_LOC, R], BF16, tag="sent_bf", name="sent_bf")
            nc.gpsimd.dma_start(out=sent_bf[:], in_=sent_in[:])  # cast f32->bf16 in DMA

            # transposed copies: hT_bf (final add), hT4 fp8 (replicated over i for MM folds),
            # sentT fp8 (cHat init + sentinel embed)
            hT_bf = cp.tile([128, RT, B_LOC], BF16, tag="hT_bf", name="hT_bf")
            hT4 = cp.tile([128, RT, S_PER_CHUNK, B_LOC], F8, tag="hT4", name="hT4")
            sentT = cp.tile([128, RT, B_LOC], F8, tag="sentT", name="sentT")
            for rb in range(RT):
                pt = ps_small.tile([128, 128], BF16, tag="pssm", name=f"pt_h_{rb}")
                nc.tensor.transpose(pt[:], h_bf[:, rb * 128:(rb + 1) * 128], ident[:])
                nc.vector.tensor_copy(hT_bf[:, rb, :], pt[:])
                nc.vector.tensor_copy(
                    hT4[:, rb, :, :],
                    pt[:].unsqueeze(1).broadcast_to([128, S_PER_CHUNK, B_LOC]))
                pt2 = ps_small.tile([128, 128], BF16, tag="pssm", name=f"pt_s_{rb}")
                nc.tensor.transpose(pt2[:], sent_bf[:, rb * 128:(rb + 1) * 128], ident[:])
                nc.vector.tensor_copy(sentT[:, rb, :], pt2[:])

            # h_eT' = 64*h_e = (h @ (64 W_h)).T + 64*b_h   [128p(a), AT, 128b] bf16
            # (fp8 DR matmuls on hT4 pairs)
            h_eT = cp.tile([128, AT, B_LOC], BF16, tag="h_eT", name="h_eT")
            for ab in range(AT):
                psh = ps_small.tile([128, B_LOC], F32, tag="pssm", name=f"psh_{ab}")
                for t in range(2):
                    nc.tensor.matmul(psh[:], w_h[:, t, :, ab * 128:(ab + 1) * 128],
                                     hT4[:, 2 * t:2 * t + 2, 0, :],
                                     start=(t == 0), stop=(t == 1), perf_mode=DR)
                nc.scalar.activation(h_eT[:, ab, :], psh[:], AF.Identity,
                                     bias=b_h64[:, ab:ab + 1], scale=1.0)

            # hA_sentT = tanh((sent_e' + h_eT')/64 + b_s)   [128p(a), AT, 128b] fp8
            hA_sentT = cp.tile([128, AT, B_LOC], F8, tag="hA_sentT", name="hA_sentT")
            for ab in range(AT):
                pss = ps_small.tile([128, B_LOC], F32, tag="pssm", name=f"pss_{ab}")
                for t in range(2):
                    nc.tensor.matmul(pss[:], w_s[:, t, :, ab * 128:(ab + 1) * 128],
                                     sentT[:, 2 * t:2 * t + 2, :],
                                     start=(t == 0), stop=(t == 1), perf_mode=DR)
                tmp = small_p.tile([128, B_LOC], F32, tag="preptmp", name=f"ptmp_{ab}")
                nc.vector.tensor_tensor(out=tmp[:], in0=pss[:], in1=h_eT[:, ab, :], op=ALU.add)
                nc.scalar.activation(hA_sentT[:, ab, :], tmp[:], AF.Tanh,
                                     bias=b_s[:, ab:ab + 1], scale=1.0 / W_C_SCALE)

            # sentinel exp'd logit -> e_sb[:, 0], and cacc init = e0 * sentT
            ps_lr0 = ps_small.tile([1, B_LOC], F32, tag="pssm", name="ps_lr0")
            for t in range(2):
                nc.tensor.matmul(ps_lr0[:], wal[:, t, :, 0:1],
                                 hA_sentT[:, 2 * t:2 * t + 2, :],
                                 start=(t == 0), stop=(t == 1), perf_mode=DR)
            e0_row = small_p.tile([1, B_LOC], BF16, tag="lrow", name="e0_row")
            nc.scalar.activation(e0_row[:], ps_lr0[:], AF.Exp, scale=1.0 / W_AL_SCALE)
            ps_ec0 = ps_small.tile([128, 2], BF16, tag="pssm", name="ps_ec0")
            nc.tensor.transpose(ps_ec0[:, 0:1], e0_row[:], ident[0:1, 0:1])
            nc.vector.tensor_copy(e_sb[:, 0:1], ps_ec0[:, 0:1])
            ps_e0rep = ps_rep.tile([128, B_LOC], F32, tag="rep", name="ps_e0rep")
            nc.tensor.matmul(ps_e0rep[:], ones_row[:], e0_row[:], start=True, stop=True)
            nc.vector.tensor_tensor(
                out=cacc[:], in0=sentT[:],
                in1=ps_e0rep[:].unsqueeze(1).broadcast_to([128, RT, B_LOC]),
                op=ALU.mult)

            prep_out.update(hT_bf=hT_bf, hT4=hT4, h_eT=h_eT)

        # ---- main x-chunk pipeline ----
        attT_chunks = {}
        hat_chunks = {}
        erep_psum = {}

        def stage_mm(c):
            nat = nat_tiles.pop(c)
            # u16 xbar transpose: attf[q, fb, i, j](u16) = nat_u16[b=j, i, fb*128+q]
            attf = attf_p.tile([128, FB, S_PER_CHUNK, 128], BF16, tag="attf", name=f"attf_{c}")
            natv = nat[:].bitcast(BF16)  # [128, 4, 1024]
            for i in range(S_PER_CHUNK):
                nc.sync.dma_start(out=attf[:, :, i, :], in_=natv[:, i, :], transpose=True)
            attf8 = attf[:].bitcast(F8)  # [128, FB, 4, 256]

            attT = attT_p.tile([128, RT, XCHUNK], F8, tag="attT", name=f"attT_{c}")
            for rb in range(RT):
                ps1 = ps_mm1.tile([128, XCHUNK], F32, tag="mm1", name=f"ps1_{c}_{rb}")
                for fb in range(FB):
                    rhs = attf8[:, fb, :, :].rearrange("p i (j two) -> p two (i j)", two=2)
                    nc.tensor.matmul(ps1[:], w_ae[:, fb, :, rb * 128:(rb + 1) * 128],
                                     rhs, start=(fb == 0), stop=(fb == FB - 1),
                                     perf_mode=DR)
                nc.scalar.activation(attT[:, rb, :], ps1[:], AF.Relu,
                                     bias=b_ae[:, rb:rb + 1], scale=1.0 / W_AE_SCALE)
            attT_chunks[c] = attT

        def stage_b_mm(c):
            attT = attT_chunks[c]
            h_eT = prep_out["h_eT"]
            # MM2 (fp8 DR) -> +h_eT' -> tanh(x/64 + b_c) -> hat fp8
            hat = hat_p.tile([128, AT, XCHUNK], F8, tag="hat", name=f"hat_{c}")
            for ab in range(AT):
                ps2 = ps_mm2.tile([128, XCHUNK], F32, tag="mm2", name=f"ps2_{c}_{ab}")
                for t in range(2):
                    nc.tensor.matmul(ps2[:], w_c[:, t, :, ab * 128:(ab + 1) * 128],
                                     attT[:, 2 * t:2 * t + 2, :],
                                     start=(t == 0), stop=(t == 1), perf_mode=DR)
                tmp = small_p.tile([128, XCHUNK], BF16, tag="hatmp", name=f"hatmp_{c}_{ab}")
                nc.vector.tensor_tensor(
                    out=tmp[:].rearrange("p (s b) -> p s b", s=S_PER_CHUNK),
                    in0=ps2[:].rearrange("p (s b) -> p s b", s=S_PER_CHUNK),
                    in1=h_eT[:, ab, :].unsqueeze(1).broadcast_to([128, S_PER_CHUNK, B_LOC]),
                    op=ALU.add)
                nc.scalar.activation(hat[:, ab, :], tmp[:], AF.Tanh,
                                     bias=b_c[:, ab:ab + 1], scale=1.0 / W_C_SCALE)
            hat_chunks[c] = hat

        def stage_logit(c):
            hat = hat_chunks.pop(c)
            # logits row (fp8 DR) -> exp -> e_row bf16
            ps_l = ps_small.tile([1, XCHUNK], F32, tag="pssm", name=f"ps_l_{c}")
            for t in range(2):
                nc.tensor.matmul(ps_l[:], wal[:, t, :, 0:1],
                                 hat[:, 2 * t:2 * t + 2, :],
                                 start=(t == 0), stop=(t == 1), perf_mode=DR)
            e_row = erow_p.tile([1, XCHUNK], BF16, tag="lrow", name=f"e_row_{c}")
            nc.scalar.activation(e_row[:], ps_l[:], AF.Exp, scale=1.0 / W_AL_SCALE)

            # e columns -> e_sb table (bf16 psum, even columns for 4B alignment)
            ps_cc = ps_small.tile([128, 2 * S_PER_CHUNK], BF16, tag="pssm", name=f"ps_cc_{c}")
            for i in range(S_PER_CHUNK):
                nc.tensor.transpose(ps_cc[:, 2 * i:2 * i + 1], e_row[:, i * 128:(i + 1) * 128],
                                    ident[0:1, 0:1])
            nc.vector.tensor_copy(
                e_sb[:, 1 + c * S_PER_CHUNK: 1 + (c + 1) * S_PER_CHUNK].unsqueeze(2),
                ps_cc[:].rearrange("p (i two) -> p i two", two=2)[:, :, 0:1])

            # broadcast e across partitions: e_rep[p, i*128+b] = e_row[i*128+b]
            ps_er = ps_rep.tile([128, XCHUNK], F32, tag="rep", name=f"ps_er_{c}")
            nc.tensor.matmul(ps_er[:], ones_row[:], e_row[:], start=True, stop=True)
            erep_psum[c] = ps_er

        def stage_chat(c):
            # cacc += sum_i attT * e_rep   (DVE, pair-tree adds)
            attT = attT_chunks.pop(c)
            ps_er = erep_psum.pop(c)
            prod = prod_p.tile([128, RT, S_PER_CHUNK, B_LOC], F32, tag="prod", name=f"prod_{c}")
            nc.vector.tensor_tensor(
                out=prod[:],
                in0=attT[:].rearrange("p rb (i j) -> p rb i j", i=S_PER_CHUNK),
                in1=ps_er[:].rearrange("p (i j) -> p i j", i=S_PER_CHUNK)
                    .unsqueeze(1).broadcast_to([128, RT, S_PER_CHUNK, B_LOC]),
                op=ALU.mult)
            t1 = prod_p.tile([128, RT, B_LOC], F32, tag="tree", name=f"t1_{c}")
            nc.vector.tensor_tensor(out=t1[:], in0=prod[:, :, 0, :], in1=prod[:, :, 1, :], op=ALU.add)
            t2 = prod_p.tile([128, RT, B_LOC], F32, tag="tree2", name=f"t2_{c}")
            nc.vector.tensor_tensor(out=t2[:], in0=prod[:, :, 2, :], in1=prod[:, :, 3, :], op=ALU.add)
            t3 = prod_p.tile([128, RT, B_LOC], F32, tag="tree3", name=f"t3_{c}")
            nc.vector.scalar_tensor_tensor(out=t3[:], in0=t1[:], scalar=0.0, in1=t2[:],
                                           op0=ALU.add, op1=ALU.add)
            nc.vector.tensor_tensor(out=cacc[:], in0=cacc[:], in1=t3[:], op=ALU.add)

        # pipeline offsets: mm(c) | b_mm(c-1) | logit(c-2) | chat(c-3)
        stage_mm(0)
        stage_in(2)
        stage_mm(1)
        stage_in(3)
        prep()
        stage_b_mm(0)
        for c in range(2, NCHUNKS + 3):
            if 0 <= c - 3 < NCHUNKS:
                stage_chat(c - 3)
            if 0 <= c - 2 < NCHUNKS:
                stage_logit(c - 2)
            if 0 <= c - 1 < NCHUNKS:
                stage_b_mm(c - 1)
            if c < NCHUNKS:
                stage_mm(c)
            if c + 2 < NCHUNKS:
                stage_in(c + 2)

        # ---- tail: normalize cHat, add h, project, store ----
        ssum = soft_p.tile([B_LOC, 1], F32, tag="soft", name="ssum")
        nc.vector.tensor_reduce(out=ssum[:], in_=e_sb[:], op=ALU.add,
                                axis=mybir.AxisListType.X)
        rinv = soft_p.tile([B_LOC, 1], F32, tag="soft", name="rinv")
        nc.vector.reciprocal(rinv[:], ssum[:])
        ps_rr = ps_small.tile([1, B_LOC], F32, tag="pssm", name="ps_rr")
        nc.tensor.transpose(ps_rr[:], rinv[:], ident_f32[:])
        rr_row = soft_p.tile([1, B_LOC], F32, tag="soft", name="rr_row")
        nc.vector.tensor_copy(rr_row[:], ps_rr[:])
        ones_f32 = soft_p.tile([1, B_LOC], F32, tag="soft2", name="ones_f32")
        nc.vector.tensor_copy(ones_f32[:], ones_row[:])
        ps_rrep = ps_rep.tile([128, B_LOC], F32, tag="rep", name="ps_rrep")
        nc.tensor.matmul(ps_rrep[:], ones_f32[:], rr_row[:], start=True, stop=True)

        hT_bf = prep_out["hT_bf"]
        attenT = cp.tile([128, RT, B_LOC], BF16, tag="attenT", name="attenT")
        nc.vector.tensor_tensor(
            out=attenT[:], in0=cacc[:],
            in1=ps_rrep[:].unsqueeze(1).broadcast_to([128, RT, B_LOC]),
            op=ALU.mult)
        nc.vector.tensor_tensor(out=attenT[:], in0=attenT[:], in1=hT_bf[:], op=ALU.add)

        for ob in range(RT):
            pso = ps_small.tile([128, B_LOC], F32, tag="pssm", name=f"pso_{ob}")
            for rb in range(RT):
                nc.tensor.matmul(pso[:], w_o[:, rb, ob * 128:(ob + 1) * 128],
                                 attenT[:, rb, :], start=(rb == 0), stop=(rb == RT - 1))
            otmp = small_p.tile([128, B_LOC], F32, tag="otmp", name=f"otmp_{ob}")
            nc.scalar.activation(otmp[:], pso[:], AF.Tanh,
                                 bias=b_o[:, ob:ob + 1], scale=1.0)
            ptb = ps_small.tile([128, 128], F32, tag="pssm", name=f"ptb_{ob}")
            nc.tensor.transpose(ptb[:], otmp[:], ident_f32[:])
            ostg = small_p.tile([128, 128], F32, tag="ostg", name=f"ostg_{ob}")
            nc.vector.tensor_copy(ostg[:], ptb[:])
            nc.gpsimd.dma_start(out=out_d[:, ob * 128:(ob + 1) * 128], in_=ostg[:])

    nc.compile()
    return nc


# ---------------- host side ----------------
_NC_CACHE = None


def _get_nc():
    global _NC_CACHE
    if _NC_CACHE is None:
        _NC_CACHE = build_nc()
    return _NC_CACHE


def prep_shared(W_ae, b_ae, W_c, b_c, W_s, b_s, W_h, b_h, W_al, b_al, W_o, b_o):
    bf = ml_dtypes.bfloat16
    f8 = ml_dtypes.float8_e4m3

    # DoubleRow pair-packed weights, f-index permuted for the u16 xbar:
    #   w_ae[p, fb, two, r] = 128 * W_ae.T[256*fb + 2*p + two, r]
    wt = np.ascontiguousarray(np.asarray(W_ae, np.float32).T * W_AE_SCALE)
    w_ae_t = np.ascontiguousarray(
        wt.reshape(FB, 128, 2, R).transpose(1, 0, 2, 3)).astype(f8)

    def pair_pack(w, scale):  # [p, t, two, n] = scale * w.T[(2t+two)*128 + p, n]
        wT = np.ascontiguousarray(np.asarray(w, np.float32).T * scale)
        return np.ascontiguousarray(
            wT.reshape(2, 2, 128, wT.shape[1]).transpose(2, 0, 1, 3)).astype(f8)

    def bt(b, nt):  # [p, t] = b[128*t + p]
        return np.ascontiguousarray(
            np.asarray(b, np.float32).reshape(nt, 128).T).astype(np.float32)

    wal_flat = np.asarray(W_al, np.float32)[0] * W_AL_SCALE  # [A]
    w_al_t = np.zeros((128, 2, 2, 16), dtype=f8)
    w_al_t[:, :, :, 0] = wal_flat.reshape(2, 2, 128).transpose(2, 0, 1).astype(f8)

    woT = np.ascontiguousarray(np.asarray(W_o, np.float32).T)
    w_o_t = np.ascontiguousarray(
        woT.reshape(RT, 128, R).transpose(1, 0, 2)).astype(bf)

    return {
        "w_ae": w_ae_t,
        "w_c": pair_pack(W_c, W_C_SCALE),
        "w_s": pair_pack(W_s, W_C_SCALE),
        "w_h": pair_pack(W_h, W_C_SCALE),
        "w_al": w_al_t,
        "w_o": w_o_t,
        "b_ae": bt(b_ae, RT),
        "b_c": bt(b_c, AT),
        "b_s": bt(b_s, AT),
        "b_h64": bt(np.asarray(b_h, np.float32) * W_C_SCALE, AT),
        "b_o": bt(b_o, RT),
        "ident": np.eye(128, dtype=bf),
        "ident_f32": np.eye(128, dtype=np.float32),
        "ones_row": np.ones((1, 128), dtype=bf),
    }


def make_in_maps(h, sentinel, att_feats, shared):
    h = np.asarray(h, np.float32)
    sentinel = np.asarray(sentinel, np.float32)
    att_feats = np.asarray(att_feats, np.float32)
    in_maps = []
    for i in range(NCORES):
        sl = slice(i * B_LOC, (i + 1) * B_LOC)
        m = dict(shared)
        m["h"] = np.ascontiguousarray(h[sl])
        m["sentinel"] = np.ascontiguousarray(sentinel[sl])
        m["att_feats"] = np.ascontiguousarray(att_feats[sl])
        in_maps.append(m)
    return in_maps


def kernel(h, sentinel, att_feats, W_ae, b_ae, W_c, b_c, W_s, b_s,
           W_h, b_h, W_al, b_al, W_o, b_o):
    shared = prep_shared(W_ae, b_ae, W_c, b_c, W_s, b_s, W_h, b_h, W_al, b_al, W_o, b_o)
    in_maps = make_in_maps(h, sentinel, att_feats, shared)
    nc = _get_nc()
    from concourse.bass_utils import run_bass_kernel_spmd
    res = run_bass_kernel_spmd(nc, in_maps, core_ids=list(range(NCORES)))
    out = np.concatenate([res.results[i]["out"] for i in range(NCORES)], axis=0)
    return np.ascontiguousarray(out.astype(np.float32))


if __name__ == "__main__":
    build_nc()
    print("built ok")
